# revision 1
# baseline (speedup 1.0000x reference)
"""Trainium2 Bass kernel for nn_AttentionGenerator (gnn_message_passing).

Reference math:
    f = einsum('oc,bctv->botv', Wf, feat) + bf          # 1x1 conv, Cout=64
    s_i = einsum('c,bctv->btv', Wa[:64], f)
    s_j = einsum('c,bctv->btv', Wa[64:], f)
    score[b,t,i,j] = s_i[b,t,i] + s_j[b,t,j] + ba
    atten = (exp(leaky_relu(score)) * A) / row_sum

Because f only enters through the two dot products, fold Wf/bf/Wa/ba on
the host into u1 = w1@Wf, u2 = w2@Wf (length-256 vectors) and the scalar
c0 = (w1+w2)@bf + ba.  The device then computes, per (b,t,v), two
channel contractions (TensorEngine), an 18x18 broadcast-add + LeakyReLU
+ exp*A + row-normalize (Vector/Scalar engines).  Memory bound: reads
151 MB of feat, writes 10.6 MB.

Sharding: pure data parallel — batch B=32 split across 8 NeuronCores
(4 batches each), tiny params replicated, no cross-core comms.
"""

import json
import numpy as np
from contextlib import ExitStack

B, Cin, T, V = 32, 256, 256, 18
NCORES = 8
BPC = B // NCORES  # batches per core
TV = T * V
PB = 128  # t-block size (partition dim)
NTB = T // PB

_cached_nc = None


def _legalize_waits_json(bir_json):
    """Split instructions carrying >1 sync wait into single-wait NoOps plus
    the original instruction.  The walrus build in this container accepts at
    most ONE sync-wait command per instruction struct; concourse's Tile
    scheduler freely attaches several.  Hoisting the extra waits onto NoOps
    immediately before the instruction (same engine stream, same position)
    preserves semantics exactly — engines execute their stream in order."""
    bir = json.loads(bir_json)
    ctr = 0
    for fn in bir.get("functions", []):
        for blk in fn.get("blocks", []):
            insts = blk.get("instructions")
            if not insts:
                continue
            out = []
            for inst in insts:
                si = inst.get("sync_info") or {}
                waits = si.get("on_wait") or []
                if len(waits) > 1:
                    for w in waits[:-1]:
                        out.append(
                            {
                                "engine": inst.get("engine"),
                                "ins": [],
                                "name": f"wsplit-{ctr}",
                                "opcode": "NoOp",
                                "outs": [],
                                "sync_info": {"on_update": [], "on_wait": [w]},
                            }
                        )
                        ctr += 1
                    si = dict(si)
                    si["on_wait"] = [waits[-1]]
                    inst = dict(inst)
                    inst["sync_info"] = si
                out.append(inst)
            blk["instructions"] = out
    return json.dumps(bir).encode()


_wait_patch_done = False


def _install_wait_legalizer():
    global _wait_patch_done
    if _wait_patch_done:
        return
    import concourse.bass_utils as bass_utils
    import concourse.bass2jax as bass2jax

    orig = bass_utils.compile_bir_kernel

    def wrapped(bir_json, tmpdir, neff_name="file.neff"):
        return orig(_legalize_waits_json(bir_json), tmpdir, neff_name)

    bass_utils.compile_bir_kernel = wrapped
    bass2jax.compile_bir_kernel = wrapped
    _wait_patch_done = True


def _build_nc():
    import concourse.bass as bass
    import concourse.mybir as mybir
    import concourse.tile as tile
    from concourse.alu_op_type import AluOpType

    f32 = mybir.dt.float32
    # feat/weights move through the PE in bf16: halves the dominant HBM
    # stream (the kernel is memory-bound) and runs the moving operand at
    # 1 cycle/row.  Accumulation stays fp32 in PSUM; quantization costs
    # ~7e-4 relative error on the output.
    bf16 = mybir.dt.bfloat16
    nc = bass.Bass(num_swdge_queues=4)
    feat = nc.dram_tensor("feat", [BPC, Cin, T, V], bf16, kind="ExternalInput")
    # wmat[k, c, o]: o-th contraction vector (u1/u2), c-chunk k of 128
    wmat = nc.dram_tensor("wmat", [2, 128, 2], bf16, kind="ExternalInput")
    amat = nc.dram_tensor("amat", [V, V], f32, kind="ExternalInput")
    cmat = nc.dram_tensor("cmat", [1, 1], f32, kind="ExternalInput")
    out = nc.dram_tensor("out", [BPC, T, V, V], f32, kind="ExternalOutput")

    with ExitStack() as ctx:
        tc = ctx.enter_context(tile.TileContext(nc))
        singles = ctx.enter_context(tc.tile_pool(name="singles", bufs=1))
        fpool = ctx.enter_context(tc.tile_pool(name="fpool", bufs=BPC * NTB))
        pspool = ctx.enter_context(tc.tile_pool(name="pspool", bufs=2, space="PSUM"))
        spool = ctx.enter_context(tc.tile_pool(name="spool", bufs=4))
        work = ctx.enter_context(tc.tile_pool(name="work", bufs=4))
        opool = ctx.enter_context(tc.tile_pool(name="opool", bufs=4))

        w_t = singles.tile([128, 2, 2], bf16)
        nc.sync.dma_start(out=w_t, in_=wmat[:, :, :].rearrange("k p o -> p k o"))
        a_bc = singles.tile([128, V, V], f32)
        nc.sync.dma_start(out=a_bc, in_=amat[:, :].partition_broadcast(128))
        c0_t = singles.tile([128, 1], f32)
        nc.sync.dma_start(out=c0_t, in_=cmat[0, :].partition_broadcast(128))

        # Absorb const-DMA waits on cheap ops so steady-state instructions
        # carry fewer sync waits (less NoOp splitting at compile).
        scratch_c = singles.tile([128, 1], f32)
        nc.vector.tensor_copy(out=scratch_c, in_=c0_t)
        scratch_a = singles.tile([128, V, V], f32)
        nc.vector.tensor_copy(out=scratch_a, in_=a_bc)

        BV = PB * V  # 2304: free size of one t-block

        def transpose_stage(st):
            """SBUF scatter s_sb[o, (t v)] -> [t, v] tiles via the ACT HWDGE
            ring (the SWDGE ring processes these 72B-row descriptors far too
            slowly, and the SP ring carries the feat prefetch stream)."""
            s1t = work.tile([128, V], f32)
            nc.gpsimd.dma_start(
                out=s1t, in_=st["s_sb"][0:1, :].rearrange("o (t v) -> o t v", v=V)
            )
            s2t = work.tile([128, V], f32)
            nc.gpsimd.dma_start(
                out=s2t, in_=st["s_sb"][1:2, :].rearrange("o (t v) -> o t v", v=V)
            )
            st["s1t"], st["s2t"] = s1t, s2t

        def head_stage(st):
            """DVE head (lag 2): score broadcast-add + LeakyReLU."""
            s1t, s2t = st["s1t"], st["s2t"]
            sc = work.tile([128, V, V], f32)
            s1b = bass.AP(
                tensor=s1t.tensor,
                offset=s1t.offset,
                ap=[s1t.ap[0], [1, V], [0, V]],
            )
            s2b = bass.AP(
                tensor=s2t.tensor,
                offset=s2t.offset,
                ap=[s2t.ap[0], [0, V], [1, V]],
            )
            # sc = (s1 + c0) + s2
            nc.vector.scalar_tensor_tensor(
                out=sc,
                in0=s1b,
                scalar=c0_t[:, :],
                in1=s2b,
                op0=AluOpType.add,
                op1=AluOpType.add,
            )
            # LeakyReLU(x) = max(x, 0.1*x); written into the per-batch
            # double-block tile so exp runs once per batch
            if st["tb"] == 0:
                lr_b_new = work.tile([128, 2, V, V], f32, tag="lr_b", name="lr_b")
                st["lr_b"] = lr_b_new
            lr_b = stages[2 * st["b"]]["lr_b"]
            nc.vector.scalar_tensor_tensor(
                out=lr_b[:, st["tb"]],
                in0=sc,
                scalar=0.1,
                in1=sc,
                op0=AluOpType.mult,
                op1=AluOpType.max,
            )

        def exp_stage(m):
            """ACT exp, one 648-element pass per batch (fewer ACT
            dispatches than per-t-block)."""
            st = stages[2 * m]
            ex_b = work.tile([128, 2, V, V], f32, tag="ex_b")
            nc.scalar.activation(
                out=ex_b, in_=st["lr_b"], func=mybir.ActivationFunctionType.Exp
            )
            st["ex_b"] = ex_b

        def tail_stage(st):
            """DVE tail: exa = ex*A, row-sum, reciprocal, normalize, into the
            per-batch output tile (one out-DMA per batch)."""
            even = stages[2 * st["b"]]
            ex = even["ex_b"][:, st["tb"]]
            exa = work.tile([128, V, V], f32)
            nc.vector.tensor_mul(out=exa, in0=ex, in1=a_bc)
            ssum = work.tile([128, V], f32)
            nc.vector.reduce_sum(out=ssum, in_=exa, axis=mybir.AxisListType.X)
            rec = work.tile([128, V], f32)
            nc.vector.reciprocal(out=rec, in_=ssum)
            if st["tb"] == 0:
                att_b_new = opool.tile([128, 2, V, V], f32, tag="att_b", name="att_b")
                even["att_b"] = att_b_new
            att_b = even["att_b"]
            rbc = bass.AP(
                tensor=rec.tensor,
                offset=rec.offset,
                ap=[rec.ap[0], [1, V], [0, V]],
            )
            nc.vector.tensor_mul(out=att_b[:, st["tb"]], in0=exa, in1=rbc)

        def out_stage(m):
            """One output DMA per batch on the ACT HWDGE ring (the SP ring's
            FIFO carries the feat prefetch stream)."""
            nc.scalar.dma_start(
                out=out[m, :, :, :].rearrange("(tb p) i j -> p tb i j", p=128),
                in_=stages[2 * m]["att_b"],
            )

        stages = []
        for b in range(BPC):
            for tb in range(NTB):
                n = len(stages)
                # feat[b, :, tb-block, :] as [c_in_chunk, chunk, t*v]
                f_t = fpool.tile([128, 2, BV], bf16)
                nc.sync.dma_start(
                    out=f_t,
                    in_=feat[
                        b, :, tb * PB : (tb + 1) * PB, :
                    ].rearrange("(k p) t v -> p k (t v)", p=128),
                )
                # s[o, t*v] = sum_c u_o[c] * feat[c, t*v]: w chunk stationary
                # (2-column LDW), feat moving in bank-aligned <=512 slices
                # (fp32r runs the moving operand at 1 cycle/row when the
                # slice is >=256), accumulated over the two c-chunks in PSUM.
                s_sb = spool.tile([2, BV], f32)
                for si, (base, widths) in enumerate(
                    ((0, (512, 512, 128)), (1152, (512, 512, 128)))
                ):
                    ps = pspool.tile([2, sum(widths)], f32, tag="ps")
                    lo = 0
                    for w in widths:
                        for k in range(2):
                            nc.tensor.matmul(
                                out=ps[:, lo : lo + w],
                                lhsT=w_t[:, k, :],
                                rhs=f_t[:, k, base + lo : base + lo + w],
                                start=(k == 0),
                                stop=(k == 1),
                            )
                        lo += w
                    nc.scalar.copy(
                        out=s_sb[:, base : base + sum(widths)], in_=ps
                    )
                stages.append({"s_sb": s_sb, "b": b, "tb": tb})
                if n >= 2:
                    transpose_stage(stages[n - 2])
                    head_stage(stages[n - 2])
                if n >= 3 and n % 2 == 1:
                    exp_stage((n - 3) // 2)
                if n >= 3:
                    tail_stage(stages[n - 3])
                if n >= 5 and (n - 5) % 2 == 0:
                    out_stage((n - 5) // 2)
        for st in stages[-2:]:
            transpose_stage(st)
            head_stage(st)
        exp_stage(BPC - 1)
        for st in stages[-3:]:
            tail_stage(st)
        for m in range(BPC - 2, BPC):
            out_stage(m)
    return nc


def _prep_params(Wf, bf, Wa, ba):
    import ml_dtypes  # noqa: F401
    w1, w2 = Wa[:64].astype(np.float64), Wa[64:].astype(np.float64)
    Wf64, bf64 = Wf.astype(np.float64), bf.astype(np.float64)
    u1 = w1 @ Wf64
    u2 = w2 @ Wf64
    c0 = float(w1 @ bf64 + w2 @ bf64 + float(ba[0]))
    import ml_dtypes

    wmat = np.stack([u1, u2], axis=-1).reshape(2, 128, 2).astype(ml_dtypes.bfloat16)
    cmat = np.full((1, 1), c0, dtype=np.float32)
    return wmat, cmat


def get_nc():
    global _cached_nc
    if _cached_nc is None:
        _cached_nc = _build_nc()
    return _cached_nc


def kernel(feat, A, Wf, bf, Wa, ba):
    _install_wait_legalizer()
    from concourse.bass_utils import run_bass_kernel_spmd

    import ml_dtypes

    feat = np.ascontiguousarray(np.asarray(feat, dtype=np.float32).astype(ml_dtypes.bfloat16))
    A = np.ascontiguousarray(np.asarray(A, dtype=np.float32))
    wmat, cmat = _prep_params(
        np.asarray(Wf, np.float32),
        np.asarray(bf, np.float32),
        np.asarray(Wa, np.float32),
        np.asarray(ba, np.float32),
    )

    nc = get_nc()
    in_maps = [
        {
            "feat": feat[i * BPC : (i + 1) * BPC],
            "wmat": wmat,
            "amat": A,
            "cmat": cmat,
        }
        for i in range(NCORES)
    ]
    res = run_bass_kernel_spmd(nc, in_maps, core_ids=list(range(NCORES)))
    return np.concatenate([r["out"] for r in res.results], axis=0)



# revision 9
# speedup vs baseline: 1.1933x; 1.1933x over previous
"""Trainium2 Bass kernel for nn_AttentionGenerator (gnn_message_passing).

Reference math:
    f = einsum('oc,bctv->botv', Wf, feat) + bf          # 1x1 conv, Cout=64
    s_i = einsum('c,bctv->btv', Wa[:64], f)
    s_j = einsum('c,bctv->btv', Wa[64:], f)
    score[b,t,i,j] = s_i[b,t,i] + s_j[b,t,j] + ba
    atten = (exp(leaky_relu(score)) * A) / row_sum

Because f only enters through the two dot products, fold Wf/bf/Wa/ba on
the host into u1 = w1@Wf, u2 = w2@Wf (length-256 vectors) and the scalar
c0 = (w1+w2)@bf + ba.

Device pipeline (memory-bound problem -> minimize HBM bytes):
  * feat is sent in fp8 e4m3 (halves the dominant HBM stream vs bf16);
    u1/u2 are scaled by 64 into the fp8 normal range, with the 1/64
    descale folded into the exp's input scale (max/mult commute with
    positive scaling, so leaky-relu can run on the scaled scores).
  * TensorE pass 1: per batch-pair, 18 DoubleRow matmuls (2 fp8
    rows/cycle) contract the 256 channels for each graph node v ->
    sT[(v,o), t] in PSUM.  The 36 stationary columns are u1/u2 shifted
    per-v, so s_i and s_j come from a single pass over feat.
  * DVE evacuates sT to SBUF as bf16, adding 64*c0/2 to every row (both
    score halves then carry half the bias).
  * TensorE pass 2: score[t, (i,j)] = s1[t,i]+s2[t,j] is linear in sT,
    so one tiny matmul per t-block against a constant 0/1 matrix G
    [36, 324] assembles the full score tile - no transposes, no DMA
    scatters, no broadcast-add.
  * DVE: exact leaky-relu (max(x, .1x)); ACT: exp (one table, loaded
    once); DVE/gpsimd: *A, row-sum, reciprocal, normalize.
  * Junk matmuls during the initial feat DMA warm the PE HAM clock gate
    (cold PE runs at 1.2 GHz; warm at 2.4).
  * Outputs are written bf16 and upcast on the host (atten in [0,1]).

Sharding: pure data parallel - batch B=32 split across 8 NeuronCores
(4 batches each), tiny params replicated, no cross-core comms.
"""

import json
import numpy as np
from contextlib import ExitStack

B, Cin, T, V = 32, 256, 256, 18
NCORES = 8
BPC = B // NCORES  # batches per core
NPAIR = BPC // 2  # batch pairs per core
SC = 64.0  # weight prescale so u1/u2 land in fp8-normal range
VV = V * V  # 324
WPAD = 80  # padded weight column pitch (DoubleRow k-tile step must be %16)

_cached_nc = None


def _legalize_waits_json(bir_json):
    """Split instructions carrying >1 sync wait into single-wait NoOps plus
    the original instruction.  The walrus build in this container accepts at
    most ONE sync-wait command per instruction struct; concourse's Tile
    scheduler freely attaches several.  Hoisting the extra waits onto NoOps
    immediately before the instruction (same engine stream, same position)
    preserves semantics exactly - engines execute their stream in order."""
    bir = json.loads(bir_json)
    ctr = 0
    for fn in bir.get("functions", []):
        for blk in fn.get("blocks", []):
            insts = blk.get("instructions")
            if not insts:
                continue
            out = []
            for inst in insts:
                si = inst.get("sync_info") or {}
                waits = si.get("on_wait") or []
                if len(waits) > 1:
                    for w in waits[:-1]:
                        out.append(
                            {
                                "engine": inst.get("engine"),
                                "ins": [],
                                "name": f"wsplit-{ctr}",
                                "opcode": "NoOp",
                                "outs": [],
                                "sync_info": {"on_update": [], "on_wait": [w]},
                            }
                        )
                        ctr += 1
                    si = dict(si)
                    si["on_wait"] = [waits[-1]]
                    inst = dict(inst)
                    inst["sync_info"] = si
                out.append(inst)
            blk["instructions"] = out
    return json.dumps(bir).encode()


_wait_patch_done = False


def _install_wait_legalizer():
    global _wait_patch_done
    if _wait_patch_done:
        return
    import concourse.bass_utils as bass_utils
    import concourse.bass2jax as bass2jax

    orig = bass_utils.compile_bir_kernel

    def wrapped(bir_json, tmpdir, neff_name="file.neff"):
        return orig(_legalize_waits_json(bir_json), tmpdir, neff_name)

    bass_utils.compile_bir_kernel = wrapped
    bass2jax.compile_bir_kernel = wrapped
    _wait_patch_done = True


def _build_nc():
    import concourse.bass as bass
    import concourse.mybir as mybir
    import concourse.tile as tile
    from concourse.alu_op_type import AluOpType

    f32 = mybir.dt.float32
    bf16 = mybir.dt.bfloat16
    fp8 = mybir.dt.float8e4
    nc = bass.Bass(num_swdge_queues=4)

    # feat[pair, p, v, kt, (b2 t)]: channel c = kt*128 + p, fp8 e4m3
    feat = nc.dram_tensor(
        "feat", [NPAIR, 128, V, 2, 2 * T], fp8, kind="ExternalInput"
    )
    # wmat[p, kt, col]: zeros except col 36 = 64*u1[kt*128+p], col 37 = 64*u2
    wmat = nc.dram_tensor("wmat", [128, 2, WPAD], fp8, kind="ExternalInput")
    # amat[(tb, i, j)]: A duplicated per t-block, bf16
    amat = nc.dram_tensor("amat", [2 * VV], bf16, kind="ExternalInput")
    # gmat[(v,o), (i,j)]: score-assembly 0/1 matrix, bf16
    gmat = nc.dram_tensor("gmat", [2 * V, VV], bf16, kind="ExternalInput")
    # cmat: SC*c0/2
    cmat = nc.dram_tensor("cmat", [1, 1], f32, kind="ExternalInput")
    out = nc.dram_tensor("out", [BPC, 128, 2 * VV], bf16, kind="ExternalOutput")

    with ExitStack() as ctx:
        tc = ctx.enter_context(tile.TileContext(nc))
        singles = ctx.enter_context(tc.tile_pool(name="singles", bufs=1))
        fpool = ctx.enter_context(tc.tile_pool(name="fpool", bufs=NPAIR))
        ps_sT = ctx.enter_context(tc.tile_pool(name="ps_sT", bufs=2, space="PSUM"))
        ps_sc = ctx.enter_context(tc.tile_pool(name="ps_sc", bufs=2, space="PSUM"))
        ps_junk = ctx.enter_context(tc.tile_pool(name="ps_junk", bufs=2, space="PSUM"))
        work = ctx.enter_context(tc.tile_pool(name="work", bufs=2))
        opool = ctx.enter_context(tc.tile_pool(name="opool", bufs=2))

        w_t = singles.tile([128, 2, WPAD], fp8)
        nc.scalar.dma_start(out=w_t, in_=wmat[:, :, :])
        a_bc = singles.tile([128, 2 * VV], bf16)
        nc.scalar.dma_start(out=a_bc, in_=amat[:].partition_broadcast(128))
        g_t = singles.tile([2 * V, VV], bf16)
        nc.scalar.dma_start(out=g_t, in_=gmat[:, :])
        c0h = singles.tile([2 * V, 1], f32)
        nc.scalar.dma_start(out=c0h, in_=cmat[0, :].partition_broadcast(2 * V))

        # feat loads: 2 per pair (v 0:9 / 9:18), dispatched up-front (SP ring)
        f_tiles = []
        for pr in range(NPAIR):
            f_t = fpool.tile([128, V, 2, 2 * T], fp8)
            nc.sync.dma_start(out=f_t[:, : V // 2], in_=feat[pr, :, : V // 2])
            nc.sync.dma_start(out=f_t[:, V // 2 :], in_=feat[pr, :, V // 2 :])
            f_tiles.append(f_t)

        # PE warm-up: junk matmuls during the first feat DMA keep the HAM
        # activity window busy so real matmuls start at 2.4 GHz.
        for wi in range(16):
            jp = ps_junk.tile([128, 128], f32)
            nc.tensor.matmul(
                out=jp,
                lhsT=a_bc[:, 0:128],
                rhs=a_bc[:, 0:128],
                start=True,
                stop=True,
            )

        def emit_pair(pr):
            f_t = f_tiles[pr]
            # --- TensorE: sT[(v,o), (b2 t)] over both batches of the pair ---
            sT_ps = ps_sT.tile([2 * V, 2 * T], f32)
            for v0 in range(V):
                nc.tensor.matmul(
                    out=sT_ps[:, :],
                    lhsT=w_t[:, :, 36 - 2 * v0 : 72 - 2 * v0],
                    rhs=f_t[:, v0, :, :],
                    start=(v0 == 0),
                    stop=(v0 == V - 1),
                    perf_mode=mybir.MatmulPerfMode.DoubleRow,
                )
            # --- DVE: evacuate to SBUF bf16, add SC*c0/2 to every row ---
            sT_sb = work.tile([2 * V, 2 * T], bf16)
            nc.vector.tensor_scalar(
                out=sT_sb,
                in0=sT_ps,
                scalar1=c0h[:, :],
                scalar2=None,
                op0=AluOpType.add,
            )
            return sT_sb

        def emit_batch(sT_sb, b, half):
            st = {"b": b}
            # --- TensorE: score[t, (i,j)] = sT.T @ G per t-block ---
            sc_ps = ps_sc.tile([128, 2, 512], f32)
            for tb in range(2):
                nc.tensor.matmul(
                    out=sc_ps[:, tb, :VV],
                    lhsT=sT_sb[:, (2 * half + tb) * 128 : (2 * half + tb + 1) * 128],
                    rhs=g_t[:, :],
                    start=True,
                    stop=True,
                )
            # --- exp(leaky(x)) = max(exp(x), exp(.1 x)): two Exp passes on
            # ACT (same table -> loaded once) + one packed-bf16 DVE max ---
            e1 = work.tile([128, 2, VV], bf16)
            nc.scalar.activation(
                out=e1, in_=sc_ps[:, :, :VV],
                func=mybir.ActivationFunctionType.Exp,
                scale=1.0 / SC,
            )
            e2 = work.tile([128, 2, VV], bf16)
            nc.scalar.activation(
                out=e2, in_=sc_ps[:, :, :VV],
                func=mybir.ActivationFunctionType.Exp,
                scale=0.1 / SC,
            )
            ex = work.tile([128, 2 * VV], bf16)
            nc.vector.tensor_tensor(
                out=ex.rearrange("p (tb x) -> p tb x", tb=2),
                in0=e1, in1=e2, op=AluOpType.max
            )
            # --- gpsimd: exa = ex * A ---
            exa = work.tile([128, 2 * VV], bf16)
            nc.gpsimd.tensor_mul(out=exa, in0=ex, in1=a_bc)
            # --- DVE: row-sum + reciprocal ---
            ssum = work.tile([128, 2 * V], f32)
            nc.vector.reduce_sum(
                out=ssum,
                in_=exa.rearrange("p (g j) -> p g j", j=V),
                axis=mybir.AxisListType.X,
            )
            rec = work.tile([128, 2 * V], f32)
            nc.vector.reciprocal(out=rec, in_=ssum)
            # --- gpsimd: normalize ---
            att = opool.tile([128, 2 * VV], bf16)
            nc.gpsimd.tensor_mul(
                out=att.rearrange("p (g j) -> p g j", j=V),
                in0=exa.rearrange("p (g j) -> p g j", j=V),
                in1=rec.unsqueeze(2).broadcast_to([128, 2 * V, V]),
            )
            st["att"] = att
            return st

        stages = []
        for pr in range(NPAIR):
            sT_sb = emit_pair(pr)
            for half in range(2):
                b = 2 * pr + half
                stages.append(emit_batch(sT_sb, b, half))
                if b >= 1:
                    prev = stages[b - 1]
                    nc.scalar.dma_start(out=out[prev["b"]], in_=prev["att"])
        last = stages[BPC - 1]
        nc.scalar.dma_start(out=out[last["b"]], in_=last["att"])
    return nc


def _prep_params(Wf, bf, Wa, ba):
    import ml_dtypes

    f8 = ml_dtypes.float8_e4m3fn
    bf16 = ml_dtypes.bfloat16
    w1, w2 = Wa[:64].astype(np.float64), Wa[64:].astype(np.float64)
    Wf64, bf64 = Wf.astype(np.float64), bf.astype(np.float64)
    u1 = (w1 @ Wf64) * SC
    u2 = (w2 @ Wf64) * SC
    c0 = float(w1 @ bf64 + w2 @ bf64 + float(ba[0]))
    wmat = np.zeros((128, 2, WPAD), dtype=f8)
    # u[kt*128 + p] at padded col 36 (u1) / 37 (u2)
    wmat[:, 0, 36] = u1[:128].astype(np.float32).astype(f8)
    wmat[:, 1, 36] = u1[128:].astype(np.float32).astype(f8)
    wmat[:, 0, 37] = u2[:128].astype(np.float32).astype(f8)
    wmat[:, 1, 37] = u2[128:].astype(np.float32).astype(f8)
    cmat = np.full((1, 1), c0 * SC / 2.0, dtype=np.float32)
    # G[(v,o), (i,j)]: score = s1[i] + s2[j] as a linear map of sT rows
    G = np.zeros((2 * V, VV), dtype=np.float32)
    for v in range(V):
        G[2 * v + 0, v * V : (v + 1) * V] = 1.0  # s1[v] -> rows i == v
        G[2 * v + 1, v::V] = 1.0  # s2[v] -> cols j == v
    gmat = G.astype(bf16)
    return wmat, cmat, gmat


def get_nc():
    global _cached_nc
    if _cached_nc is None:
        _cached_nc = _build_nc()
    return _cached_nc


def kernel(feat, A, Wf, bf, Wa, ba):
    _install_wait_legalizer()
    from concourse.bass_utils import run_bass_kernel_spmd

    import ml_dtypes

    f8 = ml_dtypes.float8_e4m3fn
    bf16 = ml_dtypes.bfloat16

    # [B, 256c, T, V] -> fp8, c=(kt,p); pairs of batches share one tile:
    # [pair, p, v, kt, (b2 t)]
    featq = np.asarray(feat, dtype=np.float32).astype(f8)
    featq = featq.reshape(B // 2, 2, 2, 128, T, V).transpose(0, 3, 5, 2, 1, 4)
    featq = np.ascontiguousarray(featq).reshape(B // 2, 128, V, 2, 2 * T)

    A2 = np.tile(np.asarray(A, np.float32).reshape(VV), 2).astype(bf16)
    wmat, cmat, gmat = _prep_params(
        np.asarray(Wf, np.float32),
        np.asarray(bf, np.float32),
        np.asarray(Wa, np.float32),
        np.asarray(ba, np.float32),
    )

    nc = get_nc()
    in_maps = [
        {
            "feat": featq[i * NPAIR : (i + 1) * NPAIR],
            "wmat": wmat,
            "amat": A2,
            "gmat": gmat,
            "cmat": cmat,
        }
        for i in range(NCORES)
    ]
    res = run_bass_kernel_spmd(nc, in_maps, core_ids=list(range(NCORES)))
    # out[b, p, (tb, i, j)] bf16 -> [b, t=(tb,p), i, j] f32
    outs = []
    for r in res.results:
        o = r["out"].astype(np.float32).reshape(BPC, 128, 2, V, V)
        outs.append(o.transpose(0, 2, 1, 3, 4).reshape(BPC, T, V, V))
    return np.concatenate(outs, axis=0)


# revision 10
# speedup vs baseline: 1.2340x; 1.0341x over previous
"""Trainium2 Bass kernel for nn_AttentionGenerator (gnn_message_passing).

Reference math:
    f = einsum('oc,bctv->botv', Wf, feat) + bf          # 1x1 conv, Cout=64
    s_i = einsum('c,bctv->btv', Wa[:64], f)
    s_j = einsum('c,bctv->btv', Wa[64:], f)
    score[b,t,i,j] = s_i[b,t,i] + s_j[b,t,j] + ba
    atten = (exp(leaky_relu(score)) * A) / row_sum

Because f only enters through the two dot products, fold Wf/bf/Wa/ba on
the host into u1 = w1@Wf, u2 = w2@Wf (length-256 vectors) and the scalar
c0 = (w1+w2)@bf + ba.

Device pipeline (memory-bound problem -> minimize HBM bytes):
  * feat is sent in fp8 e4m3 (halves the dominant HBM stream vs bf16);
    u1/u2 are scaled by 64 into the fp8 normal range, with the 1/64
    descale folded into the exp's input scale (max/mult commute with
    positive scaling, so leaky-relu can run on the scaled scores).
  * TensorE pass 1: per batch-pair, 18 DoubleRow matmuls (2 fp8
    rows/cycle) contract the 256 channels for each graph node v ->
    sT[(v,o), t] in PSUM.  The 36 stationary columns are u1/u2 shifted
    per-v, so s_i and s_j come from a single pass over feat.
  * DVE evacuates sT to SBUF as bf16, adding 64*c0/2 to every row (both
    score halves then carry half the bias).
  * TensorE pass 2: score[t, (i,j)] = s1[t,i]+s2[t,j] is linear in sT,
    so one tiny matmul per t-block against a constant 0/1 matrix G
    [36, 324] assembles the full score tile - no transposes, no DMA
    scatters, no broadcast-add.
  * DVE: exact leaky-relu (max(x, .1x)); ACT: exp (one table, loaded
    once); DVE/gpsimd: *A, row-sum, reciprocal, normalize.
  * Junk matmuls during the initial feat DMA warm the PE HAM clock gate
    (cold PE runs at 1.2 GHz; warm at 2.4).
  * Outputs are written bf16 and upcast on the host (atten in [0,1]).

Sharding: pure data parallel - batch B=32 split across 8 NeuronCores
(4 batches each), tiny params replicated, no cross-core comms.
"""

import json
import numpy as np
from contextlib import ExitStack

B, Cin, T, V = 32, 256, 256, 18
NCORES = 8
BPC = B // NCORES  # batches per core
NPAIR = BPC // 2  # batch pairs per core
SC = 64.0  # weight prescale so u1/u2 land in fp8-normal range
VV = V * V  # 324
WPAD = 80  # padded weight column pitch (DoubleRow k-tile step must be %16)

_cached_nc = None


def _legalize_waits_json(bir_json):
    """Split instructions carrying >1 sync wait into single-wait NoOps plus
    the original instruction.  The walrus build in this container accepts at
    most ONE sync-wait command per instruction struct; concourse's Tile
    scheduler freely attaches several.  Hoisting the extra waits onto NoOps
    immediately before the instruction (same engine stream, same position)
    preserves semantics exactly - engines execute their stream in order."""
    bir = json.loads(bir_json)
    ctr = 0
    for fn in bir.get("functions", []):
        for blk in fn.get("blocks", []):
            insts = blk.get("instructions")
            if not insts:
                continue
            out = []
            for inst in insts:
                si = inst.get("sync_info") or {}
                waits = si.get("on_wait") or []
                if len(waits) > 1:
                    for w in waits[:-1]:
                        out.append(
                            {
                                "engine": inst.get("engine"),
                                "ins": [],
                                "name": f"wsplit-{ctr}",
                                "opcode": "NoOp",
                                "outs": [],
                                "sync_info": {"on_update": [], "on_wait": [w]},
                            }
                        )
                        ctr += 1
                    si = dict(si)
                    si["on_wait"] = [waits[-1]]
                    inst = dict(inst)
                    inst["sync_info"] = si
                out.append(inst)
            blk["instructions"] = out
    return json.dumps(bir).encode()


_wait_patch_done = False


def _install_wait_legalizer():
    global _wait_patch_done
    if _wait_patch_done:
        return
    import concourse.bass_utils as bass_utils
    import concourse.bass2jax as bass2jax

    orig = bass_utils.compile_bir_kernel

    def wrapped(bir_json, tmpdir, neff_name="file.neff"):
        return orig(_legalize_waits_json(bir_json), tmpdir, neff_name)

    bass_utils.compile_bir_kernel = wrapped
    bass2jax.compile_bir_kernel = wrapped
    _wait_patch_done = True


def _build_nc():
    import concourse.bass as bass
    import concourse.mybir as mybir
    import concourse.tile as tile
    from concourse.alu_op_type import AluOpType

    f32 = mybir.dt.float32
    bf16 = mybir.dt.bfloat16
    fp8 = mybir.dt.float8e4
    nc = bass.Bass(num_swdge_queues=4)

    # feat[pair, p, v, kt, (b2 t)]: channel c = kt*128 + p, fp8 e4m3
    feat = nc.dram_tensor(
        "feat", [NPAIR, 128, V, 2, 2 * T], fp8, kind="ExternalInput"
    )
    # wmat[p, kt, col]: zeros except col 36 = 64*u1[kt*128+p], col 37 = 64*u2
    wmat = nc.dram_tensor("wmat", [128, 2, WPAD], fp8, kind="ExternalInput")
    # amat[(tb, i, j)]: A duplicated per t-block, bf16
    amat = nc.dram_tensor("amat", [2 * VV], bf16, kind="ExternalInput")
    # gmat[(v,o), (i,j)]: score-assembly 0/1 matrix, bf16
    gmat = nc.dram_tensor("gmat", [2 * V, VV], bf16, kind="ExternalInput")
    # cmat: SC*c0/2
    cmat = nc.dram_tensor("cmat", [1, 1], f32, kind="ExternalInput")
    out = nc.dram_tensor("out", [BPC, 128, 2 * VV], bf16, kind="ExternalOutput")

    with ExitStack() as ctx:
        tc = ctx.enter_context(tile.TileContext(nc))
        singles = ctx.enter_context(tc.tile_pool(name="singles", bufs=1))
        fpool = ctx.enter_context(tc.tile_pool(name="fpool", bufs=NPAIR))
        ps_sT = ctx.enter_context(tc.tile_pool(name="ps_sT", bufs=2, space="PSUM"))
        ps_sc = ctx.enter_context(tc.tile_pool(name="ps_sc", bufs=2, space="PSUM"))
        ps_junk = ctx.enter_context(tc.tile_pool(name="ps_junk", bufs=2, space="PSUM"))
        work = ctx.enter_context(tc.tile_pool(name="work", bufs=2))
        opool = ctx.enter_context(tc.tile_pool(name="opool", bufs=2))

        w_t = singles.tile([128, 2, WPAD], fp8)
        nc.scalar.dma_start(out=w_t, in_=wmat[:, :, :])
        a_bc = singles.tile([128, 2 * VV], bf16)
        nc.scalar.dma_start(out=a_bc, in_=amat[:].partition_broadcast(128))
        g_t = singles.tile([2 * V, VV], bf16)
        nc.scalar.dma_start(out=g_t, in_=gmat[:, :])
        c0h = singles.tile([2 * V, 1], f32)
        nc.scalar.dma_start(out=c0h, in_=cmat[0, :].partition_broadcast(2 * V))

        # feat loads: 2 per pair (v 0:9 / 9:18), dispatched up-front (SP ring)
        f_tiles = []
        for pr in range(NPAIR):
            f_t = fpool.tile([128, V, 2, 2 * T], fp8)
            nc.sync.dma_start(out=f_t[:, : V // 2], in_=feat[pr, :, : V // 2])
            nc.sync.dma_start(out=f_t[:, V // 2 :], in_=feat[pr, :, V // 2 :])
            f_tiles.append(f_t)

        # PE warm-up: junk matmuls from preamble-end until the first feat
        # tile lands keep the HAM activity window busy, so real matmuls run
        # at 2.4 GHz instead of the cold 1.2 GHz.
        jsrc = singles.tile([128, 256], bf16)
        nc.gpsimd.memset(jsrc, 0.0)
        for wi in range(56):
            jp = ps_junk.tile([128, 256], f32)
            nc.tensor.matmul(
                out=jp,
                lhsT=jsrc[:, 0:128],
                rhs=jsrc[:, :],
                start=True,
                stop=True,
            )

        def emit_pair(pr):
            f_t = f_tiles[pr]
            # --- TensorE: sT[(v,o), (b2 t)] over both batches of the pair ---
            sT_ps = ps_sT.tile([2 * V, 2 * T], f32)
            for v0 in range(V):
                nc.tensor.matmul(
                    out=sT_ps[:, :],
                    lhsT=w_t[:, :, 36 - 2 * v0 : 72 - 2 * v0],
                    rhs=f_t[:, v0, :, :],
                    start=(v0 == 0),
                    stop=(v0 == V - 1),
                    perf_mode=mybir.MatmulPerfMode.DoubleRow,
                )
            # --- DVE: evacuate to SBUF bf16, add SC*c0/2 to every row ---
            sT_sb = work.tile([2 * V, 2 * T], bf16)
            nc.vector.tensor_scalar(
                out=sT_sb,
                in0=sT_ps,
                scalar1=c0h[:, :],
                scalar2=None,
                op0=AluOpType.add,
            )
            return sT_sb

        def emit_batch(sT_sb, b, half):
            st = {"b": b}
            # --- TensorE: score[t, (i,j)] = sT.T @ G per t-block ---
            sc_ps = ps_sc.tile([128, 2, 512], f32)
            for tb in range(2):
                nc.tensor.matmul(
                    out=sc_ps[:, tb, :VV],
                    lhsT=sT_sb[:, (2 * half + tb) * 128 : (2 * half + tb + 1) * 128],
                    rhs=g_t[:, :],
                    start=True,
                    stop=True,
                )
            # --- exp(leaky(x)) = max(exp(x), exp(.1 x)): two Exp passes on
            # ACT (same table -> loaded once) + one packed-bf16 DVE max ---
            e1 = work.tile([128, 2, VV], bf16)
            nc.scalar.activation(
                out=e1, in_=sc_ps[:, :, :VV],
                func=mybir.ActivationFunctionType.Exp,
                scale=1.0 / SC,
            )
            e2 = work.tile([128, 2, VV], bf16)
            nc.scalar.activation(
                out=e2, in_=sc_ps[:, :, :VV],
                func=mybir.ActivationFunctionType.Exp,
                scale=0.1 / SC,
            )
            ex = work.tile([128, 2 * VV], bf16)
            nc.vector.tensor_tensor(
                out=ex.rearrange("p (tb x) -> p tb x", tb=2),
                in0=e1, in1=e2, op=AluOpType.max
            )
            # --- DVE: exa = ex * A (packed bf16 2x mode) ---
            exa = work.tile([128, 2 * VV], bf16)
            nc.vector.tensor_mul(out=exa, in0=ex, in1=a_bc)
            # --- DVE: row-sum + reciprocal ---
            ssum = work.tile([128, 2 * V], f32)
            nc.vector.reduce_sum(
                out=ssum,
                in_=exa.rearrange("p (g j) -> p g j", j=V),
                axis=mybir.AxisListType.X,
            )
            rec = work.tile([128, 2 * V], f32)
            nc.vector.reciprocal(out=rec, in_=ssum)
            # --- gpsimd: normalize ---
            att = opool.tile([128, 2 * VV], bf16)
            nc.gpsimd.tensor_mul(
                out=att.rearrange("p (g j) -> p g j", j=V),
                in0=exa.rearrange("p (g j) -> p g j", j=V),
                in1=rec.unsqueeze(2).broadcast_to([128, 2 * V, V]),
            )
            st["att"] = att
            return st

        stages = []
        for pr in range(NPAIR):
            sT_sb = emit_pair(pr)
            for half in range(2):
                b = 2 * pr + half
                stages.append(emit_batch(sT_sb, b, half))
                if b >= 1:
                    prev = stages[b - 1]
                    nc.sync.dma_start(out=out[prev["b"]], in_=prev["att"])
        last = stages[BPC - 1]
        nc.sync.dma_start(out=out[last["b"]], in_=last["att"])
    return nc


def _prep_params(Wf, bf, Wa, ba):
    import ml_dtypes

    f8 = ml_dtypes.float8_e4m3fn
    bf16 = ml_dtypes.bfloat16
    w1, w2 = Wa[:64].astype(np.float64), Wa[64:].astype(np.float64)
    Wf64, bf64 = Wf.astype(np.float64), bf.astype(np.float64)
    u1 = (w1 @ Wf64) * SC
    u2 = (w2 @ Wf64) * SC
    c0 = float(w1 @ bf64 + w2 @ bf64 + float(ba[0]))
    wmat = np.zeros((128, 2, WPAD), dtype=f8)
    # u[kt*128 + p] at padded col 36 (u1) / 37 (u2)
    wmat[:, 0, 36] = u1[:128].astype(np.float32).astype(f8)
    wmat[:, 1, 36] = u1[128:].astype(np.float32).astype(f8)
    wmat[:, 0, 37] = u2[:128].astype(np.float32).astype(f8)
    wmat[:, 1, 37] = u2[128:].astype(np.float32).astype(f8)
    cmat = np.full((1, 1), c0 * SC / 2.0, dtype=np.float32)
    # G[(v,o), (i,j)]: score = s1[i] + s2[j] as a linear map of sT rows
    G = np.zeros((2 * V, VV), dtype=np.float32)
    for v in range(V):
        G[2 * v + 0, v * V : (v + 1) * V] = 1.0  # s1[v] -> rows i == v
        G[2 * v + 1, v::V] = 1.0  # s2[v] -> cols j == v
    gmat = G.astype(bf16)
    return wmat, cmat, gmat


def get_nc():
    global _cached_nc
    if _cached_nc is None:
        _cached_nc = _build_nc()
    return _cached_nc


def kernel(feat, A, Wf, bf, Wa, ba):
    _install_wait_legalizer()
    from concourse.bass_utils import run_bass_kernel_spmd

    import ml_dtypes

    f8 = ml_dtypes.float8_e4m3fn
    bf16 = ml_dtypes.bfloat16

    # [B, 256c, T, V] -> fp8, c=(kt,p); pairs of batches share one tile:
    # [pair, p, v, kt, (b2 t)]
    featq = np.asarray(feat, dtype=np.float32).astype(f8)
    featq = featq.reshape(B // 2, 2, 2, 128, T, V).transpose(0, 3, 5, 2, 1, 4)
    featq = np.ascontiguousarray(featq).reshape(B // 2, 128, V, 2, 2 * T)

    A2 = np.tile(np.asarray(A, np.float32).reshape(VV), 2).astype(bf16)
    wmat, cmat, gmat = _prep_params(
        np.asarray(Wf, np.float32),
        np.asarray(bf, np.float32),
        np.asarray(Wa, np.float32),
        np.asarray(ba, np.float32),
    )

    nc = get_nc()
    in_maps = [
        {
            "feat": featq[i * NPAIR : (i + 1) * NPAIR],
            "wmat": wmat,
            "amat": A2,
            "gmat": gmat,
            "cmat": cmat,
        }
        for i in range(NCORES)
    ]
    res = run_bass_kernel_spmd(nc, in_maps, core_ids=list(range(NCORES)))
    # out[b, p, (tb, i, j)] bf16 -> [b, t=(tb,p), i, j] f32
    outs = []
    for r in res.results:
        o = r["out"].astype(np.float32).reshape(BPC, 128, 2, V, V)
        outs.append(o.transpose(0, 2, 1, 3, 4).reshape(BPC, T, V, V))
    return np.concatenate(outs, axis=0)
